# revision 11
# baseline (speedup 1.0000x reference)
"""Trainium2 Bass kernel for a dense transformer decoder block (B=2, S=2048,
D=1024, H=16, F=4096).  kernel(**inputs) -> (out, attn_weights), FULL shapes.

Sharding (8 NeuronCores, SPMD, no collectives): sequence-sharded over query
rows with causal load balancing.  Core i owns query chunk i of batch 0 and
chunk 7-i of batch 1 (256 rows each -> 512 q rows/core) and computes K/V for
exactly 2304 kv positions.  Attention runs against a host-built additive
mask (causal + batch-match), so the SPMD program is identical on all cores.
KV is processed in two halves of 1152 to fit SBUF.  The attention
probabilities are emitted unnormalized and normalized on the host; attn_out
is normalized on-chip via a PE-broadcast reciprocal before out_proj.
Matmuls run in float32r (measured ~1.5e-4 rel err, bf16 speed).
"""

import numpy as np
import ml_dtypes

import concourse.bass as bass
from concourse import bacc
import concourse.mybir as mybir
from concourse.tile import TileContext
from concourse.bass_utils import run_bass_kernel_spmd
from concourse.masks import make_identity

B, S, D, H, F = 2, 2048, 1024, 16, 4096
NC = 8
QR = S // NC          # 256
RQ = 2 * QR           # 512 q rows/core
KV = S + QR           # 2304 kv rows/core
KVH = KV // 2         # 1152
EPS = 1e-5
P = 128
DC = D // P           # 8
FC = F // P           # 32
NKB = KVH // P        # 9
KTS = [(0, 512), (512, 512), (1024, 128)]   # k-tiles within a half
F32 = mybir.dt.float32
F32R = mybir.dt.float32r
BF16 = mybir.dt.bfloat16
AF = mybir.ActivationFunctionType
OP = mybir.AluOpType
NEG8 = -8.0e9
GELU = AF.Gelu   # swappable for CoreSim (Gelu unimplemented there)

_CACHE = {}


def _bcast_ap(ap_1d, parts=P):
    return bass.AP(tensor=ap_1d.tensor, offset=ap_1d.offset,
                   ap=[[0, parts]] + list(ap_1d.ap))


def _ln(nc, pool, x_ap, g_bc, b_bc, out_ap, eps_t):
    stats = pool.tile([P, 2, 6], F32, tag="ln_stats")
    xr = x_ap.rearrange("p (s f) -> p s f", s=2)
    for s in range(2):
        nc.vector.bn_stats(stats[:, s], xr[:, s])
    mv = pool.tile([P, 2], F32, tag="ln_mv")
    nc.vector.bn_aggr(mv, stats)
    rstd = pool.tile([P, 1], F32, tag="ln_rstd")
    nc.scalar.activation(rstd, mv[:, 1:2], AF.Sqrt, bias=eps_t, scale=1.0)
    nc.vector.reciprocal(rstd, rstd)
    nc.vector.tensor_scalar(out_ap, x_ap, scalar1=mv[:, 0:1], scalar2=rstd,
                            op0=OP.subtract, op1=OP.mult)
    nc.vector.tensor_tensor(out_ap, out_ap, g_bc, OP.mult)
    nc.vector.tensor_tensor(out_ap, out_ap, b_bc, OP.add)


def build_nc():
    nc = bacc.Bacc("TRN2", target_bir_lowering=False, debug=False)

    xkv = nc.dram_tensor("xkv", [KV, D], F32, kind="ExternalInput")
    mask8 = nc.dram_tensor("mask8", [RQ, KV], BF16, kind="ExternalInput")
    sel = nc.dram_tensor("sel", [1, 2 * P], F32, kind="ExternalInput")
    w_in = {}
    for nm, shp in [("wq", [D, D]), ("wk", [D, D]), ("wv", [D, D]),
                    ("wo", [D, D]), ("w1", [D, F]), ("w2", [F, D])]:
        w_in[nm] = nc.dram_tensor(nm, shp, F32, kind="ExternalInput")
    v_in = {}
    for nm in ["bq", "bk", "bv", "bo", "ln1_g", "ln1_b", "b2", "ln2_g", "ln2_b"]:
        v_in[nm] = nc.dram_tensor(nm, [D], F32, kind="ExternalInput")
    v_in["b1"] = nc.dram_tensor("b1", [F], F32, kind="ExternalInput")

    out_p = nc.dram_tensor("out_p", [RQ, D], F32, kind="ExternalOutput")
    aw_p = nc.dram_tensor("aw_p", [H, RQ, KV], F32, kind="ExternalOutput")
    rsc = nc.dram_tensor("rsc", [64, P], F32)

    with TileContext(nc) as tc:
        with tc.tile_pool(name="const", bufs=1) as const, \
             tc.tile_pool(name="big", bufs=1) as big:
            ident = const.tile([P, P], F32)
            make_identity(nc, ident)
            eps_t = const.tile([P, 1], F32)
            nc.vector.memset(eps_t, EPS)
            sel_sb = const.tile([1, 2, P], F32, tag="sel")
            nc.sync.dma_start(sel_sb, sel[:, :].rearrange("a (b c) -> a b c", b=2))
            bq_sb = const.tile([P, DC], F32, tag="bq")
            nc.sync.dma_start(bq_sb, v_in["bq"].ap().rearrange("(c p) -> p c", p=P))
            bk_sb = const.tile([P, DC], F32, tag="bk")
            nc.sync.dma_start(bk_sb, v_in["bk"].ap().rearrange("(c p) -> p c", p=P))
            b1_sb = const.tile([P, FC], F32, tag="b1")
            nc.sync.dma_start(b1_sb, v_in["b1"].ap().rearrange("(c p) -> p c", p=P))
            bc_names = ["bv", "bo", "b2", "ln1_g", "ln1_b", "ln2_g", "ln2_b"]
            bc_all = const.tile([P, len(bc_names), D], F32, tag="bc_all")
            bc = {}
            for j, nm in enumerate(bc_names):
                nc.sync.dma_start(bc_all[:, j], _bcast_ap(v_in[nm].ap()))
                bc[nm] = bc_all[:, j]
            qT = big.tile([P, DC, RQ], F32R, tag="qT")
            aoT = big.tile([P, DC, RQ], F32, tag="aoT")
            acc = big.tile([P, RQ // P, H, 6], F32, tag="acc")

            # ================ Q projection (q rows = xkv[1792:2304]) ==========
            with tc.tile_pool(name="qw", bufs=1) as qwp, \
                 tc.tile_pool(name="qx", bufs=2) as qxp, \
                 tc.tile_pool(name="qps", bufs=2, space="PSUM") as qpj:
                wq_r = qwp.tile([P, DC, D], F32R, tag="w", name="w_q")
                nc.gpsimd.dma_start(wq_r, w_in["wq"].rearrange("(c p) n -> p c n", p=P))
                for (qo, qn) in [(KV - RQ, 256), (KV - RQ + 256, 256)]:
                    xtq = qxp.tile([P, DC, 256], F32R, tag="xT", name="xtq")
                    for ss in range(qn // P):
                        xinq = qxp.tile([P, D], F32, tag="xin", name="xinq")
                        nc.sync.dma_start(xinq, xkv[qo + ss * P: qo + ss * P + P, :])
                        for dc in range(DC):
                            tpq = qpj.tile([P, P], F32, tag="tp", name="tpq")
                            nc.tensor.transpose(tpq, xinq[:, dc * P:(dc + 1) * P], ident)
                            nc.vector.tensor_copy(xtq[:, dc, ss * P:(ss + 1) * P], tpq)
                    for do in range(DC):
                        psq = qpj.tile([P, 256], F32, tag="psk", name="psq")
                        for dc in range(DC):
                            nc.tensor.matmul(psq[:, :qn], wq_r[:, dc, do * P:(do + 1) * P],
                                             xtq[:, dc, :qn], start=(dc == 0), stop=(dc == DC - 1))
                        nc.scalar.activation(qT[:, do, qo - (KV - RQ): qo - (KV - RQ) + qn],
                                             psq[:, :qn], AF.Identity,
                                             bias=bq_sb[:, do:do + 1], scale=1.0)

            # ================ per-half: K/V projection, attention =============
            for half in range(2):
                hoff = half * KVH
                with tc.tile_pool(name=f"kv{half}", bufs=1) as hpool:
                    kT = hpool.tile([P, DC, KVH], F32R, tag="kT")
                    vna = hpool.tile([P, NKB, D], BF16, tag="vna")

                    with tc.tile_pool(name=f"pw{half}", bufs=1) as wpool, \
                         tc.tile_pool(name=f"px{half}", bufs=2) as xpool, \
                         tc.tile_pool(name=f"pp{half}", bufs=2, space="PSUM") as pj:

                        def load_w(wname):
                            w_r = wpool.tile([P, DC, D], F32R, tag="w", name=f"w_{wname}{half}")
                            nc.gpsimd.dma_start(w_r, w_in[wname].rearrange("(c p) n -> p c n", p=P))
                            return w_r

                        def make_xt(ro, rn):
                            xt = xpool.tile([P, DC, 256], F32R, tag="xT", name=f"xt{half}")
                            for ss in range(rn // P):
                                xin = xpool.tile([P, D], F32, tag="xin", name=f"xin{half}")
                                nc.sync.dma_start(
                                    xin, xkv[hoff + ro + ss * P: hoff + ro + ss * P + P, :])
                                for dc in range(DC):
                                    tp = pj.tile([P, P], F32, tag="tp")
                                    nc.tensor.transpose(tp, xin[:, dc * P:(dc + 1) * P], ident)
                                    nc.vector.tensor_copy(xt[:, dc, ss * P:(ss + 1) * P], tp)
                            return xt

                        blocks = [(i * 256, 256) for i in range(4)] + [(1024, 128)]
                        # K pass
                        wk_r = load_w("wk")
                        for (ro, rn) in blocks:
                            xt = make_xt(ro, rn)
                            for do in range(DC):
                                psk = pj.tile([P, 256], F32, tag="psk")
                                for dc in range(DC):
                                    nc.tensor.matmul(psk[:, :rn], wk_r[:, dc, do * P:(do + 1) * P],
                                                     xt[:, dc, :rn], start=(dc == 0), stop=(dc == DC - 1))
                                nc.scalar.activation(kT[:, do, ro:ro + rn], psk[:, :rn],
                                                     AF.Identity, bias=bk_sb[:, do:do + 1], scale=1.0)
                        # V pass
                        wv_r = load_w("wv")
                        for (ro, rn) in blocks:
                            xt = make_xt(ro, rn)
                            for ss in range(rn // P):
                                for do in range(2):
                                    psv = pj.tile([P, 512], F32, tag="psv")
                                    for dc in range(DC):
                                        nc.tensor.matmul(psv, xt[:, dc, ss * P:(ss + 1) * P],
                                                         wv_r[:, dc, do * 512:(do + 1) * 512],
                                                         start=(dc == 0), stop=(dc == DC - 1))
                                    nc.vector.tensor_tensor(
                                        vna[:, (ro + ss * P) // P, do * 512:(do + 1) * 512],
                                        psv, bc["bv"][:, do * 512:(do + 1) * 512], OP.add)
                    # -------- attention over this half's kv --------
                    with tc.tile_pool(name=f"at{half}", bufs=6) as apool, \
                         tc.tile_pool(name=f"ab{half}", bufs=3) as bpool, \
                         tc.tile_pool(name=f"am{half}", bufs=1) as mpool, \
                         tc.tile_pool(name=f"as{half}", bufs=2, space="PSUM") as psc, \
                         tc.tile_pool(name=f"att{half}", bufs=3, space="PSUM") as pst, \
                         tc.tile_pool(name=f"av{half}", bufs=2, space="PSUM") as psv_:
                        mask_sb = mpool.tile([P, RQ // P, KVH], BF16, tag="mask", name=f"mask{half}")
                        nc.sync.dma_start(
                            mask_sb,
                            mask8[:, hoff:hoff + KVH].rearrange("(b p) k -> p b k", p=P))
                        for qb in range(RQ // P):
                            for h in range(H):
                                po = (h % 2) * 64
                                dch = h // 2
                                qT_h = qT[po:po + 64, dch, qb * P:(qb + 1) * P]
                                exs = []
                                for ti, (ko, kn) in enumerate(KTS):
                                    ps = psc.tile([P, 512], F32, tag="sc")
                                    nc.tensor.matmul(ps[:, :kn], qT_h,
                                                     kT[po:po + 64, dch, ko:ko + kn],
                                                     start=True, stop=True)
                                    ms = apool.tile([P, 512], F32, tag="ms")
                                    nc.vector.tensor_tensor(
                                        ms[:, :kn], ps[:, :kn],
                                        mask_sb[:, qb, ko: ko + kn], OP.add)
                                    ex = apool.tile([P, 512], F32, tag="ex")
                                    nc.scalar.activation(
                                        ex[:, :kn], ms[:, :kn], AF.Exp, bias=0.0, scale=0.125,
                                        accum_out=acc[:, qb, h, half * 3 + ti: half * 3 + ti + 1])
                                    nc.sync.dma_start(
                                        aw_p[h, qb * P:(qb + 1) * P, hoff + ko: hoff + ko + kn],
                                        ex[:, :kn])
                                    exs.append(ex)
                                pvp = psv_.tile([64, P], F32, tag="pv")
                                for kb in range(NKB):
                                    ti = kb // 4 if kb < 8 else 2
                                    ko = KTS[ti][0]
                                    tp = pst.tile([P, P], F32, tag="tp")
                                    nc.tensor.transpose(
                                        tp, exs[ti][:, kb * P - ko:(kb + 1) * P - ko], ident)
                                    pbf = bpool.tile([P, P], BF16, tag="pbf")
                                    nc.vector.tensor_copy(pbf, tp)
                                    nc.tensor.matmul(pvp, vna[:, kb, h * 64:(h + 1) * 64],
                                                     pbf, start=(kb == 0), stop=(kb == NKB - 1))
                                dst = aoT[po:po + 64, dch, qb * P:(qb + 1) * P]
                                if half == 0:
                                    nc.vector.tensor_copy(dst, pvp)
                                else:
                                    nc.vector.tensor_tensor(dst, dst, pvp, OP.add)

            # ============== normalize attn_out via PE-broadcast recip =========
            mid = tc.alloc_tile_pool(name="mid", bufs=1)
            h_nat = mid.tile([P, RQ // P, D], F32, tag="h")
            hT = mid.tile([P, DC, RQ], F32R, tag="hT")
            o2 = tc.alloc_tile_pool(name="o2", bufs=1)
            aoT_r = o2.tile([P, DC, RQ], F32R, tag="aoTr")
            with tc.tile_pool(name="nrm", bufs=1) as npool, \
                 tc.tile_pool(name="nps", bufs=2, space="PSUM") as nps:
                asum = npool.tile([P, RQ // P, H], F32, tag="asum")
                nc.vector.reduce_sum(asum, acc, axis=mybir.AxisListType.X)
                nc.vector.reciprocal(asum, asum)
                rt = nps.tile([P, P], F32, tag="rt")
                nc.tensor.transpose(rt[:64, :], asum.rearrange("p a b -> p (a b)"), ident)
                rsm = npool.tile([64, P], F32, tag="rsm")
                nc.vector.tensor_copy(rsm, rt[:64, :])
                nc.sync.dma_start(rsc[:, :], rsm)
                rsm1 = npool.tile([1, 64 * P], F32, tag="rsm1")
                nc.sync.dma_start(rsm1, rsc[:, :].rearrange("a b -> (a b)")[None, :])
                rT = npool.tile([P, DC, RQ], F32, tag="rT")
                for dc in range(DC):
                    for qb in range(RQ // P):
                        pr = nps.tile([P, P], F32, tag="pr")
                        for hh in range(2):
                            hidx = qb * H + dc * 2 + hh
                            nc.tensor.matmul(pr, sel_sb[0:1, hh, :],
                                             rsm1[0:1, hidx * P:(hidx + 1) * P],
                                             start=(hh == 0), stop=(hh == 1))
                        nc.vector.tensor_copy(rT[:, dc, qb * P:(qb + 1) * P], pr)
                nc.vector.tensor_tensor(aoT, aoT, rT, OP.mult)
                nc.vector.tensor_copy(aoT_r, aoT)

            # ============== out_proj + residual + LN1 + hT ====================
            with tc.tile_pool(name="oo", bufs=1) as opool, \
                 tc.tile_pool(name="ow", bufs=1) as owp, \
                 tc.tile_pool(name="ops", bufs=4, space="PSUM") as ops:
                wo_r = owp.tile([P, DC, D], F32R, tag="wor")
                nc.gpsimd.dma_start(wo_r, w_in["wo"].rearrange("(c p) n -> p c n", p=P))
                xq_sb = opool.tile([P, RQ // P, D], F32, tag="xq")
                nc.sync.dma_start(xq_sb, xkv[KV - RQ:, :].rearrange("(b p) d -> p b d", p=P))
                for rs in range(RQ // P):
                    for do in range(2):
                        pso = ops.tile([P, 512], F32, tag="pso")
                        for dc in range(DC):
                            nc.tensor.matmul(pso, aoT_r[:, dc, rs * P:(rs + 1) * P],
                                             wo_r[:, dc, do * 512:(do + 1) * 512],
                                             start=(dc == 0), stop=(dc == DC - 1))
                        hsl = h_nat[:, rs, do * 512:(do + 1) * 512]
                        nc.vector.tensor_tensor(hsl, pso, bc["bo"][:, do * 512:(do + 1) * 512],
                                                OP.add)
                        nc.vector.tensor_tensor(hsl, hsl, xq_sb[:, rs, do * 512:(do + 1) * 512],
                                                OP.add)
                    _ln(nc, opool, h_nat[:, rs, :], bc["ln1_g"], bc["ln1_b"],
                        h_nat[:, rs, :], eps_t)
                    for dc in range(DC):
                        tp = ops.tile([P, P], F32, tag="tp2")
                        nc.tensor.transpose(tp, h_nat[:, rs, dc * P:(dc + 1) * P], ident)
                        nc.vector.tensor_copy(hT[:, dc, rs * P:(rs + 1) * P], tp)

            o2.release()

            # ============== FFN ==============================================
            ffp = tc.alloc_tile_pool(name="ffp", bufs=1)
            ff1T = ffp.tile([P, FC, RQ], F32R, tag="ff1T")
            with tc.tile_pool(name="fw", bufs=3) as fwp, \
                 tc.tile_pool(name="fps", bufs=4, space="PSUM") as fps:
                for fc in range(FC):
                    w1r = fwp.tile([P, DC, P], F32R, tag="w1r")
                    nc.gpsimd.dma_start(
                        w1r, w_in["w1"][:, fc * P:(fc + 1) * P].rearrange("(c p) f -> p c f", p=P))
                    psf = fps.tile([P, 512], F32, tag="psf")
                    for dc in range(DC):
                        nc.tensor.matmul(psf, w1r[:, dc], hT[:, dc, :],
                                         start=(dc == 0), stop=(dc == DC - 1))
                    nc.scalar.activation(ff1T[:, fc, :], psf, GELU,
                                         bias=b1_sb[:, fc:fc + 1], scale=1.0)

            with tc.tile_pool(name="f2", bufs=2) as f2p, \
                 tc.tile_pool(name="f2o", bufs=1) as f2o, \
                 tc.tile_pool(name="f2ps", bufs=1, space="PSUM") as f2ps:
                ps2 = [[f2ps.tile([P, 512], F32, tag=f"ps2_{rs}_{do}", name=f"ps2_{rs}_{do}")
                        for do in range(2)] for rs in range(RQ // P)]
                for fc in range(FC):
                    w2r = f2p.tile([P, D], F32R, tag="w2r")
                    nc.gpsimd.dma_start(w2r, w_in["w2"][fc * P:(fc + 1) * P, :])
                    for rs in range(RQ // P):
                        for do in range(2):
                            nc.tensor.matmul(ps2[rs][do], ff1T[:, fc, rs * P:(rs + 1) * P],
                                             w2r[:, do * 512:(do + 1) * 512],
                                             start=(fc == 0), stop=(fc == FC - 1))
                for rs in range(RQ // P):
                    out_sb = f2p.tile([P, D], F32, tag="osb")
                    for do in range(2):
                        osl = out_sb[:, do * 512:(do + 1) * 512]
                        nc.vector.tensor_tensor(osl, ps2[rs][do],
                                                bc["b2"][:, do * 512:(do + 1) * 512], OP.add)
                        nc.vector.tensor_tensor(osl, osl, h_nat[:, rs, do * 512:(do + 1) * 512],
                                                OP.add)
                    _ln(nc, f2o, out_sb, bc["ln2_g"], bc["ln2_b"], out_sb, eps_t)
                    nc.sync.dma_start(out_p[rs * P:(rs + 1) * P, :], out_sb)
            ffp.release()
            mid.release()

    nc.finalize()
    return nc


def _get_nc():
    if "nc" not in _CACHE:
        _CACHE["nc"] = build_nc()
    return _CACHE["nc"]


def _prep_core(i, x, cm):
    len0 = QR * (i + 1)
    len1 = QR * (NC - i)
    q0 = slice(QR * i, QR * (i + 1))
    q1 = slice(QR * (NC - 1 - i), QR * (NC - i))
    xkv = np.concatenate([x[0, :len0 - QR], x[1, :len1 - QR],
                          x[0, q0], x[1, q1]], axis=0)
    kvpos = np.concatenate([np.arange(len0 - QR), np.arange(len1 - QR),
                            np.arange(q0.start, q0.stop),
                            np.arange(q1.start, q1.stop)])
    kvbatch = np.concatenate([np.zeros(len0 - QR, np.int8), np.ones(len1 - QR, np.int8),
                              np.zeros(QR, np.int8), np.ones(QR, np.int8)])
    qpos = np.concatenate([np.arange(q0.start, q0.stop), np.arange(q1.start, q1.stop)])
    qbatch = np.concatenate([np.zeros(QR, np.int8), np.ones(QR, np.int8)])
    same = qbatch[:, None] == kvbatch[None, :]
    full = 8.0 * cm[qpos[:, None], kvpos[None, :]]
    m8 = np.where(same, full, np.float32(NEG8)).astype(ml_dtypes.bfloat16)
    return np.ascontiguousarray(xkv), m8, (len0, len1, q0, q1)


def kernel(x, causal_mask, wq, bq, wk, bk, wv, bv, wo, bo,
           ln1_g, ln1_b, w1, b1, w2, b2, ln2_g, ln2_b):
    x = np.asarray(x, np.float32)
    cm = np.asarray(causal_mask, np.float32)[0, 0]
    nc = _get_nc()

    sel = np.zeros((1, 2 * P), np.float32)
    sel[0, :64] = 1.0
    sel[0, P + 64:] = 1.0
    shared = dict(
        wq=np.asarray(wq, np.float32), wk=np.asarray(wk, np.float32),
        wv=np.asarray(wv, np.float32), wo=np.asarray(wo, np.float32),
        w1=np.asarray(w1, np.float32), w2=np.asarray(w2, np.float32),
        bq=np.asarray(bq, np.float32), bk=np.asarray(bk, np.float32),
        bv=np.asarray(bv, np.float32), bo=np.asarray(bo, np.float32),
        b1=np.asarray(b1, np.float32), b2=np.asarray(b2, np.float32),
        ln1_g=np.asarray(ln1_g, np.float32), ln1_b=np.asarray(ln1_b, np.float32),
        ln2_g=np.asarray(ln2_g, np.float32), ln2_b=np.asarray(ln2_b, np.float32),
        sel=sel)

    in_maps, meta = [], []
    for i in range(NC):
        xkv, m8, mi = _prep_core(i, x, cm)
        in_maps.append(dict(shared, xkv=xkv, mask8=m8))
        meta.append(mi)

    res = run_bass_kernel_spmd(nc, in_maps, core_ids=list(range(NC)))

    out = np.zeros((B, S, D), np.float32)
    aw = np.zeros((B, H, S, S), np.float32)
    for r, (len0, len1, q0, q1) in zip(res.results, meta):
        op = r["out_p"]
        ap = r["aw_p"]
        ap = ap / np.maximum(ap.sum(axis=2, keepdims=True), 1e-30)
        out[0, q0] = op[:QR]
        out[1, q1] = op[QR:]
        segs = [(0, len0 - QR, 0, 0), (len0 - QR, len1 - QR, 1, 0),
                (KV - RQ, QR, 0, len0 - QR), (KV - QR, QR, 1, len1 - QR)]
        for (c0, clen, b, g0) in segs:
            if clen <= 0:
                continue
            qs, qloc = (q0, slice(0, QR)) if b == 0 else (q1, slice(QR, RQ))
            aw[b, :, qs, g0:g0 + clen] = ap[:, qloc, c0:c0 + clen]
    return out, aw
